# revision 17
# baseline (speedup 1.0000x reference)
"""AgentAttention fused kernel for trn2 (8 NeuronCores).

Math (per batch b, head h):
  q_ag   = agent_tokens[0,h] @ Wq.T + bq                  [A, hs]   (host)
  q_eff  = (q_ag @ Wk) / sqrt(hs)                         [A, hs]   (host)
  c_q    = (q_ag @ bk) / sqrt(hs)                         [A]       (host)
  s1     = q_eff @ k.T + c_q                              [A, N]
  p1     = softmax(s1, axis=-1)
  c1     = (p1 @ v) @ Wv.T + bv                           [A, hs]
  s2     = (q / sqrt(hs)) @ c1.T                          [N, A]
  p2     = softmax(s2, axis=-1)        -> output #2 (scores2)
  c2     = p2 @ c1                                        [N, hs]
  out    = concat_h(c2) @ Wo.T + bo                       [N, DM]   -> output #1

Sharding: core c handles batch b=c//2, n-rows [(c%2)*2048, +2048) for
stage 2 + out projection.  Stage 1 (tiny A=64 output per head) is
computed for all 16 heads of batch b on both of its cores (duplicated,
no collectives).

On-chip layouts are "transposed world": feature dims on partitions, n on
the free axis, so every heavy matmul streams 512-wide at float32r rate.
Head pairs (2j, 2j+1) occupy partitions 0:64 / 64:128; per-head weights
are packed BLOCK-DIAGONALLY into [128,128] lhsT tiles so one K=128
matmul computes both heads with its PSUM output at partition 0 (the
walrus verifier requires matmul outputs to start at partition 0).
"""

import numpy as np

import concourse.bass as bass
import concourse.bacc as bacc
import concourse.mybir as mybir
import concourse.tile as tile
from concourse.bass_utils import run_bass_kernel_spmd

B, NH, N, HS, A, DM = 4, 16, 4096, 64, 64, 1024
NCORES = 8
NLOC = N // 2            # rows per core in stage 2
NG = NH // 2             # head-pair groups
NT1 = N // 128           # stage-1 n-chunks (32)
NS1 = N // 512           # stage-1 slabs (8)
NT2 = NLOC // 128        # stage-2 n-tiles (16)
NS2 = NLOC // 512        # stage-2 slabs (4)
NOT = DM // 128          # out-projection o-tiles (8)

F32 = mybir.dt.float32
F32R = mybir.dt.float32r
AF = mybir.ActivationFunctionType

# dtype for the u = p1@v matmul operands (fp32r is exact-ish but 4 cyc/row
# at N=128; fp16 is 1 cyc/row at ~5e-4 relative error)
U_DT = mybir.dt.float32r


def r(ap):
    """View an fp32 AP as float32r (full-rate fp32 matmul)."""
    if ap.dtype == F32R:
        return ap
    return ap.bitcast(F32R)


def mm_dt(ap):
    """Matmul-operand view for U_DT tensors."""
    return ap


def build_program():
    nc = bacc.Bacc()

    # ---- inputs (per core) ----
    kT_d = nc.declare_dram_parameter("kT", [NG, 128, N], F32R, False)
    v_d = nc.declare_dram_parameter("v", [NG, 128, NT1, 128], U_DT, False)
    qT_d = nc.declare_dram_parameter("qT", [NG, 128, NLOC], F32R, False)
    WoT_d = nc.declare_dram_parameter("WoT", [128, NOT, DM], F32R, False)
    qeT_d = nc.declare_dram_parameter("qeT", [NG, 128, 128], F32R, False)
    cq_d = nc.declare_dram_parameter("cq", [128, NG], F32, False)
    Wv_d = nc.declare_dram_parameter("Wvbd", [128, 128], F32R, False)
    bv_d = nc.declare_dram_parameter("bv", [128, 1], F32, False)
    bo_d = nc.declare_dram_parameter("bo", [128, NOT], F32, False)
    id_d = nc.declare_dram_parameter("ident", [128, 128], F32R, False)
    ob_d = nc.declare_dram_parameter("onesb", [128, 128], F32R, False)

    # ---- outputs (per core) ----
    outT_d = nc.declare_dram_parameter("outT", [NOT, 128, NLOC], F32, True)
    sc2_d = nc.declare_dram_parameter("sc2", [NG, 128, NT2, 2, A], F32, True)

    with tile.TileContext(nc) as tc:
        with (
            tc.tile_pool(name="const", bufs=1) as constp,
            tc.tile_pool(name="c1", bufs=1) as c1p,
        ):
            ident = constp.tile([128, 128], F32R)
            nc.sync.dma_start(out=ident, in_=id_d[:])
            qe_sb = constp.tile([128, NG, 128], F32R)
            nc.sync.dma_start(out=qe_sb, in_=qeT_d[:].rearrange("g p a -> p g a"))
            cq_sb = constp.tile([128, NG], F32)
            nc.sync.dma_start(out=cq_sb, in_=cq_d[:])
            wv_sb = constp.tile([128, 128], F32R)
            nc.sync.dma_start(out=wv_sb, in_=Wv_d[:])
            bv_sb = constp.tile([128, 1], F32)
            nc.sync.dma_start(out=bv_sb, in_=bv_d[:])
            ob_sb = constp.tile([128, 128], F32R)
            nc.sync.dma_start(out=ob_sb, in_=ob_d[:])

            # stage-1 products, used by stage 2 (block-diagonal, pre-zeroed)
            c1bd_sb = c1p.tile([128, NG, 128], F32R)   # diag: c1T per head
            c1nbd_sb = c1p.tile([128, NG, 128], F32R)  # diag: c1 per head
            nc.vector.memset(c1bd_sb.bitcast(F32), 0.0)

            # ================= stage 1 =================
            with (
                tc.tile_pool(name="kt", bufs=2) as ktp,
                tc.tile_pool(name="vt", bufs=2) as vp,
                tc.tile_pool(name="es1", bufs=2) as es1p,
                tc.tile_pool(name="es1t", bufs=2) as es1tp,
                tc.tile_pool(name="s1_small", bufs=2) as smallp,
                tc.tile_pool(name="ps_s1", bufs=2, space="PSUM") as ps_s1p,
                tc.tile_pool(name="ps_tr", bufs=2, space="PSUM") as ps_trp,
                tc.tile_pool(name="ps_u", bufs=2, space="PSUM") as ps_up,
                tc.tile_pool(name="ps_c1", bufs=1, space="PSUM") as ps_c1p,
            ):
                for g in range(NG):
                    kt = ktp.tile([128, N], F32R)
                    nc.sync.dma_start(out=kt, in_=kT_d[g])
                    vt = vp.tile([128, NT1, 128], U_DT)
                    nc.sync.dma_start(out=vt, in_=v_d[g])

                    exp_s1 = es1p.tile([128, N], U_DT)
                    rs = smallp.tile([128, NS1], F32, tag="rs")
                    for s in range(NS1):
                        sl = bass.ts(s, 512)
                        ps = ps_s1p.tile([128, 512], F32)
                        nc.tensor.matmul(ps, r(qe_sb[:, g]), r(kt[:, sl]))
                        # exp(s1 + c_q), fused row-sum accumulation
                        nc.scalar.activation(
                            exp_s1[:, sl], ps, AF.Exp,
                            bias=cq_sb[:, g : g + 1], scale=1.0,
                            accum_out=rs[:, s : s + 1],
                        )
                    # softmax denominators for both heads of the pair
                    rs1 = smallp.tile([128, 1], F32, tag="rs1")
                    nc.vector.tensor_reduce(
                        rs1, rs, axis=mybir.AxisListType.X, op=mybir.AluOpType.add
                    )
                    rec1 = smallp.tile([128, 1], F32, tag="rec1")
                    nc.vector.reciprocal(rec1, rs1)

                    # transpose exp_s1 -> [n, 2a] chunks (4 chunks per psum bank)
                    es1t = es1tp.tile([128, NS1, 512], U_DT)
                    for tb in range(NS1):
                        pst = ps_trp.tile([128, 512], U_DT)
                        for tq in range(4):
                            t = tb * 4 + tq
                            nc.tensor.matmul(
                                pst[:, bass.ts(tq, 128)].bitcast(U_DT),
                                exp_s1[:, bass.ts(t, 128)],
                                ident.bitcast(U_DT),
                                is_transpose=True,
                            )
                        nc.vector.tensor_copy(es1t[:, tb], pst)

                    # u = p1_unnorm @ v  (accumulate over 32 chunks)
                    # block structure: out diag blocks = uA, uB; off-diag junk
                    psu = ps_up.tile([128, 128], F32)
                    for t in range(NT1):
                        tb, tq = t // 4, t % 4
                        nc.tensor.matmul(
                            psu, mm_dt(es1t[:, tb, bass.ts(tq, 128)]),
                            mm_dt(vt[:, t]),
                            start=(t == 0), stop=(t == NT1 - 1),
                        )
                    # normalize rows of u by softmax denom
                    u_sb = smallp.tile([128, 128], F32R, tag="u")
                    nc.scalar.mul(u_sb, psu, rec1)

                    # uT (diag blocks transpose in place)
                    psut = ps_c1p.tile([128, 128], F32, tag="small")
                    nc.tensor.matmul(r(psut), r(u_sb), r(ident), is_transpose=True)
                    ut_sb = smallp.tile([128, 128], F32R, tag="ut")
                    nc.vector.tensor_copy(ut_sb, psut)

                    # c1T = Wv @ uT + bv   (block-diag Wv kills uT's junk rows;
                    # junk columns remain -> copy only diag blocks into the
                    # pre-zeroed c1bd tile)
                    psc1 = ps_c1p.tile([128, 128], F32, tag="small")
                    nc.tensor.matmul(psc1, r(wv_sb), r(ut_sb))
                    nc.scalar.activation(
                        c1bd_sb[0:64, g, 0:64], psc1[0:64, 0:64], AF.Identity,
                        bias=bv_sb[0:64], scale=1.0,
                    )
                    nc.scalar.activation(
                        c1bd_sb[64:128, g, 64:128], psc1[64:128, 64:128],
                        AF.Identity, bias=bv_sb[64:128], scale=1.0,
                    )

                    # c1 natural (block-diag transposes in place, off-diag 0)
                    psc1n = ps_c1p.tile([128, 128], F32, tag="small")
                    nc.tensor.matmul(
                        r(psc1n), r(c1bd_sb[:, g]), r(ident), is_transpose=True
                    )
                    nc.vector.tensor_copy(c1nbd_sb[:, g], psc1n)

            # ================= stage 2 =================
            with tc.tile_pool(name="c2t", bufs=1) as c2tp:
                c2T_sb = c2tp.tile([128, NG, NLOC], F32R)
                with (
                    tc.tile_pool(name="qt", bufs=2) as qtp,
                    tc.tile_pool(name="es2", bufs=2) as es2p,
                    tc.tile_pool(name="denb", bufs=2) as denbp,
                    tc.tile_pool(name="s2nt", bufs=2) as s2ntp,
                    tc.tile_pool(name="s2small", bufs=2) as s2smallp,
                    tc.tile_pool(name="ps_s2", bufs=2, space="PSUM") as ps_s2p,
                    tc.tile_pool(name="ps_den", bufs=2, space="PSUM") as ps_denp,
                    tc.tile_pool(name="ps_c2", bufs=2, space="PSUM") as ps_c2p,
                    tc.tile_pool(name="ps_tr2", bufs=2, space="PSUM") as ps_tr2p,
                ):
                    for j in range(NG):
                        qt = qtp.tile([128, NLOC], F32R)
                        nc.sync.dma_start(out=qt, in_=qT_d[j])
                        exps2 = es2p.tile([128, NLOC], F32R)
                        recb = denbp.tile([128, NLOC], F32, tag="recb")
                        s2nt = s2ntp.tile([128, NLOC], F32R)
                        for s in range(NS2):
                            sl = bass.ts(s, 512)
                            ps2 = ps_s2p.tile([128, 512], F32)
                            nc.tensor.matmul(ps2, r(c1bd_sb[:, j]), r(qt[:, sl]))
                            nc.scalar.activation(exps2[:, sl], ps2, AF.Exp)
                            # per-head softmax denominators, broadcast to the
                            # 64 partitions of each half by a block-ones matmul
                            psden = ps_denp.tile([128, 512], F32)
                            nc.tensor.matmul(psden, ob_sb, r(exps2[:, sl]))
                            nc.vector.reciprocal_approx_fast(
                                recb[:, sl], psden
                            )
                            nc.vector.tensor_mul(
                                s2nt[:, sl], exps2[:, sl], recb[:, sl]
                            )

                            # c2T for this slab (block-diag c1n; rows = m-chunk)
                            psc2 = ps_c2p.tile([128, 512], F32)
                            nc.tensor.matmul(
                                psc2, r(c1nbd_sb[:, j]), r(s2nt[:, sl])
                            )
                            nc.vector.tensor_copy(c2T_sb[:, j, sl], psc2)

                            # scores2 natural layout -> DRAM
                            pst2 = ps_tr2p.tile([128, 512], F32)
                            for tq in range(4):
                                t = s * 4 + tq
                                nc.tensor.matmul(
                                    r(pst2[:, bass.ts(tq, 128)]),
                                    r(s2nt[:, bass.ts(t, 128)]), r(ident),
                                    is_transpose=True,
                                )
                            s2n_sb = s2smallp.tile([128, 512], F32, tag="s2n")
                            nc.scalar.copy(s2n_sb, pst2)
                            nc.sync.dma_start(
                                out=sc2_d[j, :, bass.ts(s, 4)],
                                in_=s2n_sb.rearrange(
                                    "p (t e a) -> p t e a", t=4, e=2
                                ),
                            )

                # ================= out projection =================
                with (
                    tc.tile_pool(name="wot", bufs=1) as wotp,
                    tc.tile_pool(name="outsb", bufs=3) as outp,
                    tc.tile_pool(name="ps_o", bufs=2, space="PSUM") as ps_op,
                ):
                    wot_sb = wotp.tile([128, NOT, DM], F32R)
                    nc.sync.dma_start(out=wot_sb, in_=WoT_d[:])
                    bo_sb = wotp.tile([128, NOT], F32)
                    nc.sync.dma_start(out=bo_sb, in_=bo_d[:])
                    for ot in range(NOT):
                        for s in range(NS2):
                            sl = bass.ts(s, 512)
                            pso = ps_op.tile([128, 512], F32)
                            for j in range(NG):
                                nc.tensor.matmul(
                                    pso,
                                    r(wot_sb[:, j, bass.ts(ot, 128)]),
                                    r(c2T_sb[:, j, sl]),
                                    start=(j == 0), stop=(j == NG - 1),
                                )
                            osb = outp.tile([128, 512], F32)
                            nc.scalar.activation(
                                osb, pso, AF.Identity,
                                bias=bo_sb[:, ot : ot + 1], scale=1.0,
                            )
                            nc.sync.dma_start(out=outT_d[ot, :, sl], in_=osb)

    nc.compile()
    return nc


_program_cache = {}


def get_program():
    if "nc" not in _program_cache:
        _program_cache["nc"] = build_program()
    return _program_cache["nc"]


def make_in_maps(q, k, v, Wq, bq, Wk, bk, Wv, bv, Wo, bo, agent_tokens):
    """Host-side sharding/packing. All inputs np.float32 full tensors."""
    u_np = mybir.dt.np(U_DT)
    scale = np.float32(1.0 / np.sqrt(HS))
    q_ag = agent_tokens[0] @ Wq.T + bq            # [NH, A, HS]
    q_eff = (q_ag @ Wk) * scale                   # [NH, A, HS]
    c_q = (q_ag @ bk) * scale                     # [NH, A]

    # block-diagonal q_effT: [g, 0:64, 0:64] = q_eff[2g].T etc.
    qeT = np.zeros((NG, 128, 128), dtype=np.float32)
    for g in range(NG):
        qeT[g, 0:64, 0:64] = q_eff[2 * g].T
        qeT[g, 64:128, 64:128] = q_eff[2 * g + 1].T
    cq_p = np.ascontiguousarray(c_q.reshape(NG, 128).T).astype(np.float32)
    WoT = np.ascontiguousarray(
        Wo.T.reshape(NOT, 128, DM).transpose(1, 0, 2)
    ).astype(np.float32)
    Wvbd = np.zeros((128, 128), dtype=np.float32)
    Wvbd[0:64, 0:64] = Wv.T
    Wvbd[64:128, 64:128] = Wv.T
    bv_p = np.tile(bv.reshape(1, HS), (2, 1)).reshape(128, 1).astype(np.float32)
    bo_p = np.ascontiguousarray(bo.reshape(NOT, 128).T).astype(np.float32)
    ident = np.eye(128, dtype=np.float32)
    onesb = np.zeros((128, 128), dtype=np.float32)
    onesb[0:64, 0:64] = 1.0
    onesb[64:128, 64:128] = 1.0

    in_maps = []
    for c in range(NCORES):
        b = c // 2
        nlo = (c % 2) * NLOC
        kT_c = np.ascontiguousarray(
            k[b].transpose(0, 2, 1).reshape(NG, 128, N)
        ).astype(np.float32)
        # v pairs: [g, p, t, 0:64] = v[2g, t*128+p, :], [64:128] = v[2g+1]
        v_r = v[b].reshape(NH, NT1, 128, HS)       # [h, t, p, d]
        v_c = np.empty((NG, 128, NT1, 128), dtype=u_np)
        v_c[:, :, :, 0:64] = v_r[0::2].transpose(0, 2, 1, 3)
        v_c[:, :, :, 64:128] = v_r[1::2].transpose(0, 2, 1, 3)
        qT_c = np.ascontiguousarray(
            (q[b, :, nlo : nlo + NLOC, :] * scale)
            .transpose(0, 2, 1)
            .reshape(NG, 128, NLOC)
        ).astype(np.float32)
        in_maps.append(
            {
                "kT": kT_c,
                "v": v_c,
                "qT": qT_c,
                "WoT": WoT,
                "qeT": qeT,
                "cq": cq_p,
                "Wvbd": Wvbd,
                "bv": bv_p,
                "bo": bo_p,
                "ident": ident,
                "onesb": onesb,
            }
        )
    return in_maps


def assemble_outputs(results):
    """results: list of 8 dicts with 'outT' [NOT,128,NLOC] and
    'sc2' [NG,128,NT2,2,A]. Returns (output, scores2)."""
    output = np.empty((B, N, DM), dtype=np.float32)
    scores2 = np.empty((B, NH, N, A), dtype=np.float32)
    for c in range(NCORES):
        b = c // 2
        nlo = (c % 2) * NLOC
        outT = results[c]["outT"]                  # [NOT, 128, NLOC]
        output[b, nlo : nlo + NLOC, :] = outT.reshape(DM, NLOC).T
        sc2 = results[c]["sc2"]                    # [NG, 128, NT2, 2, A]
        scores2[b, :, nlo : nlo + NLOC, :] = (
            sc2.transpose(0, 3, 2, 1, 4).reshape(NH, NLOC, A)
        )
    return output, scores2


def kernel(**inputs):
    inputs = {k_: np.asarray(v_, dtype=np.float32) for k_, v_ in inputs.items()}
    nc = get_program()
    in_maps = make_in_maps(**inputs)
    res = run_bass_kernel_spmd(nc, in_maps, list(range(NCORES)))
    return assemble_outputs(res.results)


# revision 24
# speedup vs baseline: 1.0266x; 1.0266x over previous
"""AgentAttention fused kernel for trn2 (8 NeuronCores).

Math (per batch b, head h):
  q_ag   = agent_tokens[0,h] @ Wq.T + bq                  [A, hs]   (host)
  q_eff  = (q_ag @ Wk) / sqrt(hs)                         [A, hs]   (host)
  c_q    = (q_ag @ bk) / sqrt(hs)                         [A]       (host)
  s1     = q_eff @ k.T + c_q                              [A, N]
  p1     = softmax(s1, axis=-1)
  c1     = (p1 @ v) @ Wv.T + bv                           [A, hs]
  s2     = (q / sqrt(hs)) @ c1.T                          [N, A]
  p2     = softmax(s2, axis=-1)        -> output #2 (scores2)
  c2     = p2 @ c1                                        [N, hs]
  out    = concat_h(c2) @ Wo.T + bo                       [N, DM]   -> output #1

Sharding: core c handles batch b=c//2, n-rows [(c%2)*2048, +2048) for
stage 2 + out projection.  Stage 1 (tiny A=64 output per head) is
computed for all 16 heads of batch b on both of its cores (duplicated,
no collectives).

On-chip layouts are "transposed world": feature dims on partitions, n on
the free axis, so every heavy matmul streams 512-wide at float32r rate.
Head pairs (2j, 2j+1) occupy partitions 0:64 / 64:128; per-head weights
are packed BLOCK-DIAGONALLY into [128,128] lhsT tiles so one K=128
matmul computes both heads with its PSUM output at partition 0 (the
walrus verifier requires matmul outputs to start at partition 0).
"""

import numpy as np

import concourse.bass as bass
import concourse.bacc as bacc
import concourse.mybir as mybir
import concourse.tile as tile
from concourse.bass_utils import run_bass_kernel_spmd

B, NH, N, HS, A, DM = 4, 16, 4096, 64, 64, 1024
NCORES = 8
NLOC = N // 2            # rows per core in stage 2
NG = NH // 2             # head-pair groups
NT1 = N // 128           # stage-1 n-chunks (32)
NS1 = N // 512           # stage-1 slabs (8)
NT2 = NLOC // 128        # stage-2 n-tiles (16)
NS2 = NLOC // 512        # stage-2 slabs (4)
NOT = DM // 128          # out-projection o-tiles (8)
NGO = NG // 2            # head-pair groups OWNED per core in stage 1 (4)

F32 = mybir.dt.float32
F32R = mybir.dt.float32r
AF = mybir.ActivationFunctionType

# dtype for the u = p1@v matmul operands (fp32r is exact-ish but 4 cyc/row
# at N=128; fp16 is 1 cyc/row at ~5e-4 relative error)
U_DT = mybir.dt.float16
C2_DT = mybir.dt.float16


def r(ap):
    """View an fp32 AP as float32r (full-rate fp32 matmul)."""
    if ap.dtype == F32R:
        return ap
    return ap.bitcast(F32R)


def mm_dt(ap):
    """Matmul-operand view for U_DT tensors."""
    return ap


def build_program():
    nc = bacc.Bacc()

    # ---- inputs (per core) ----
    kT_d = nc.declare_dram_parameter("kT", [NGO, 128, N], F32R, False)
    v_d = nc.declare_dram_parameter("v", [NGO, 128, NT1, 128], U_DT, False)
    qT_d = nc.declare_dram_parameter("qT", [NG, 128, NLOC], F32R, False)
    WoT_d = nc.declare_dram_parameter("WoT", [128, NOT, DM], C2_DT, False)
    qeT_d = nc.declare_dram_parameter("qeT", [NGO, 128, 128], F32R, False)
    cq_d = nc.declare_dram_parameter("cq", [128, NGO], F32, False)
    Wv_d = nc.declare_dram_parameter("Wvbd", [128, 128], F32R, False)
    bv_d = nc.declare_dram_parameter("bv", [128, 1], F32, False)
    bo_d = nc.declare_dram_parameter("bo", [128, NOT], F32, False)
    id_d = nc.declare_dram_parameter("ident", [128, 128], F32R, False)
    ob_d = nc.declare_dram_parameter("onesb", [128, 128], F32R, False)
    idh_d = nc.declare_dram_parameter("identh", [128, 128], U_DT, False)

    # ---- outputs (per core) ----
    outT_d = nc.declare_dram_parameter("outT", [NOT, 128, NLOC], F32, True)
    sc2_d = nc.declare_dram_parameter("sc2", [NG, 128, NT2, 2, A], F32, True)

    with tile.TileContext(nc) as tc:
        with (
            tc.tile_pool(name="const", bufs=1) as constp,
            tc.tile_pool(name="c1", bufs=1) as c1p,
        ):
            ident = constp.tile([128, 128], F32R)
            nc.sync.dma_start(out=ident, in_=id_d[:])
            qe_sb = constp.tile([128, NGO, 128], F32R)
            nc.sync.dma_start(out=qe_sb, in_=qeT_d[:].rearrange("g p a -> p g a"))
            cq_sb = constp.tile([128, NGO], F32)
            nc.sync.dma_start(out=cq_sb, in_=cq_d[:])
            wv_sb = constp.tile([128, 128], F32R)
            nc.sync.dma_start(out=wv_sb, in_=Wv_d[:])
            bv_sb = constp.tile([128, 1], F32)
            nc.sync.dma_start(out=bv_sb, in_=bv_d[:])
            ob_sb = constp.tile([128, 128], F32R)
            nc.sync.dma_start(out=ob_sb, in_=ob_d[:])
            identh = constp.tile([128, 128], U_DT)
            nc.sync.dma_start(out=identh, in_=idh_d[:])

            # stage-1 products: local (own NGO pairs), then AllGathered
            # into the full NG-pair block-diagonal tiles for stage 2
            c1loc_sb = c1p.tile([128, NGO, 2, 128], F32R)
            nc.vector.memset(c1loc_sb.bitcast(F32), 0.0)
            c1bd_sb = c1p.tile([128, NG, 128], F32R)   # diag: c1T per head
            c1nbd_sb = c1p.tile([128, NG, 128], F32R)  # diag: c1 per head

            # ================= stage 1 =================
            with (
                tc.tile_pool(name="kt", bufs=2) as ktp,
                tc.tile_pool(name="vt", bufs=2) as vp,
                tc.tile_pool(name="es1", bufs=2) as es1p,
                tc.tile_pool(name="es1t", bufs=2) as es1tp,
                tc.tile_pool(name="s1_small", bufs=2) as smallp,
                tc.tile_pool(name="ps_s1", bufs=2, space="PSUM") as ps_s1p,
                tc.tile_pool(name="ps_tr", bufs=2, space="PSUM") as ps_trp,
                tc.tile_pool(name="ps_u", bufs=2, space="PSUM") as ps_up,
                tc.tile_pool(name="ps_c1", bufs=1, space="PSUM") as ps_c1p,
            ):
                for g in range(NGO):
                    kt = ktp.tile([128, N], F32R)
                    nc.sync.dma_start(out=kt, in_=kT_d[g])
                    vt = vp.tile([128, NT1, 128], U_DT)
                    nc.sync.dma_start(out=vt, in_=v_d[g])

                    exp_s1 = es1p.tile([128, N], U_DT)
                    rs = smallp.tile([128, NS1], F32, tag="rs")
                    for s in range(NS1):
                        sl = bass.ts(s, 512)
                        ps = ps_s1p.tile([128, 512], F32)
                        nc.tensor.matmul(ps, r(qe_sb[:, g]), r(kt[:, sl]))
                        # exp(s1 + c_q), fused row-sum accumulation
                        nc.scalar.activation(
                            exp_s1[:, sl], ps, AF.Exp,
                            bias=cq_sb[:, g : g + 1], scale=1.0,
                            accum_out=rs[:, s : s + 1],
                        )
                    # softmax denominators for both heads of the pair
                    rs1 = smallp.tile([128, 1], F32, tag="rs1")
                    nc.vector.tensor_reduce(
                        rs1, rs, axis=mybir.AxisListType.X, op=mybir.AluOpType.add
                    )
                    rec1 = smallp.tile([128, 1], F32, tag="rec1")
                    nc.vector.reciprocal(rec1, rs1)

                    # transpose exp_s1 -> [n, 2a] chunks (4 chunks per psum bank)
                    es1t = es1tp.tile([128, NS1, 512], U_DT)
                    for tb in range(NS1):
                        pst = ps_trp.tile([128, 512], U_DT)
                        for tq in range(4):
                            t = tb * 4 + tq
                            nc.tensor.matmul(
                                pst[:, bass.ts(tq, 128)],
                                exp_s1[:, bass.ts(t, 128)],
                                identh,
                                is_transpose=True,
                            )
                        nc.vector.tensor_copy(es1t[:, tb], pst)

                    # u = p1_unnorm @ v  (accumulate over 32 chunks)
                    # block structure: out diag blocks = uA, uB; off-diag junk
                    psu = ps_up.tile([128, 128], F32)
                    for t in range(NT1):
                        tb, tq = t // 4, t % 4
                        nc.tensor.matmul(
                            psu, mm_dt(es1t[:, tb, bass.ts(tq, 128)]),
                            mm_dt(vt[:, t]),
                            start=(t == 0), stop=(t == NT1 - 1),
                        )
                    # normalize rows of u by softmax denom
                    u_sb = smallp.tile([128, 128], F32R, tag="u")
                    nc.scalar.mul(u_sb, psu, rec1)

                    # uT (diag blocks transpose in place)
                    psut = ps_c1p.tile([128, 128], F32, tag="small")
                    nc.tensor.matmul(r(psut), r(u_sb), r(ident), is_transpose=True)
                    ut_sb = smallp.tile([128, 128], F32R, tag="ut")
                    nc.vector.tensor_copy(ut_sb, psut)

                    # c1T = Wv @ uT + bv   (block-diag Wv kills uT's junk rows;
                    # junk columns remain -> copy only diag blocks into the
                    # pre-zeroed c1bd tile)
                    psc1 = ps_c1p.tile([128, 128], F32, tag="small")
                    nc.tensor.matmul(psc1, r(wv_sb), r(ut_sb))
                    nc.scalar.activation(
                        c1loc_sb[0:64, g, 0, 0:64], psc1[0:64, 0:64],
                        AF.Identity, bias=bv_sb[0:64], scale=1.0,
                    )
                    nc.scalar.activation(
                        c1loc_sb[64:128, g, 0, 64:128], psc1[64:128, 64:128],
                        AF.Identity, bias=bv_sb[64:128], scale=1.0,
                    )

                    # c1 natural (block-diag transposes in place, off-diag 0)
                    psc1n = ps_c1p.tile([128, 128], F32, tag="small")
                    nc.tensor.matmul(
                        r(psc1n), r(c1loc_sb[:, g, 0]), r(ident),
                        is_transpose=True,
                    )
                    nc.vector.tensor_copy(c1loc_sb[:, g, 1], psc1n)

                # ---- AllGather c1 across the batch's core pair ----
                with tc.tile_pool(name="ccdram", bufs=1, space="DRAM") as dramp:
                    cc_in = dramp.tile([128, NGO, 2, 128], F32R)
                    cc_out = dramp.tile([2, 128, NGO, 2, 128], F32R)
                    nc.sync.dma_start(out=cc_in[:], in_=c1loc_sb)
                    nc.gpsimd.collective_compute(
                        "AllGather",
                        mybir.AluOpType.bypass,
                        replica_groups=[[0, 1], [2, 3], [4, 5], [6, 7]],
                        ins=[cc_in.opt()],
                        outs=[cc_out.opt()],
                    )
                    for rk in range(2):
                        gs = slice(rk * NGO, rk * NGO + NGO)
                        nc.sync.dma_start(
                            out=c1bd_sb[:, gs, :], in_=cc_out[rk, :, :, 0, :]
                        )
                        nc.sync.dma_start(
                            out=c1nbd_sb[:, gs, :], in_=cc_out[rk, :, :, 1, :]
                        )

            # ================= stage 2 =================
            with tc.tile_pool(name="c2t", bufs=1) as c2tp:
                c2T_sb = c2tp.tile([128, NG, NLOC], C2_DT)
                with (
                    tc.tile_pool(name="qt", bufs=2) as qtp,
                    tc.tile_pool(name="es2", bufs=2) as es2p,
                    tc.tile_pool(name="denb", bufs=2) as denbp,
                    tc.tile_pool(name="s2small", bufs=2) as s2smallp,
                    tc.tile_pool(name="ps_s2", bufs=2, space="PSUM") as ps_s2p,
                    tc.tile_pool(name="ps_den", bufs=2, space="PSUM") as ps_denp,
                    tc.tile_pool(name="ps_c2", bufs=2, space="PSUM") as ps_c2p,
                    tc.tile_pool(name="ps_tr2", bufs=2, space="PSUM") as ps_tr2p,
                ):
                    for j in range(NG):
                        qt = qtp.tile([128, NLOC], F32R)
                        nc.sync.dma_start(out=qt, in_=qT_d[j])
                        exps2 = es2p.tile([128, NLOC], F32R)
                        recb = denbp.tile([128, NLOC], F32, tag="recb")
                        s2nt = exps2
                        for s in range(NS2):
                            sl = bass.ts(s, 512)
                            ps2 = ps_s2p.tile([128, 512], F32)
                            nc.tensor.matmul(ps2, r(c1bd_sb[:, j]), r(qt[:, sl]))
                            nc.scalar.activation(exps2[:, sl], ps2, AF.Exp)
                            # per-head softmax denominators, broadcast to the
                            # 64 partitions of each half by a block-ones matmul
                            psden = ps_denp.tile([128, 512], F32)
                            nc.tensor.matmul(psden, ob_sb, r(exps2[:, sl]))
                            nc.vector.reciprocal_approx_fast(
                                recb[:, sl], psden
                            )
                            nc.vector.tensor_mul(
                                s2nt[:, sl], exps2[:, sl], recb[:, sl]
                            )

                            # c2T for this slab (block-diag c1n; rows = m-chunk)
                            psc2 = ps_c2p.tile([128, 512], F32)
                            nc.tensor.matmul(
                                psc2, r(c1nbd_sb[:, j]), r(s2nt[:, sl])
                            )
                            nc.vector.tensor_copy(c2T_sb[:, j, sl], psc2)

                            # scores2 natural layout -> DRAM
                            pst2 = ps_tr2p.tile([128, 512], F32)
                            for tq in range(4):
                                t = s * 4 + tq
                                nc.tensor.matmul(
                                    r(pst2[:, bass.ts(tq, 128)]),
                                    r(s2nt[:, bass.ts(t, 128)]), r(ident),
                                    is_transpose=True,
                                )
                            s2n_sb = s2smallp.tile([128, 512], F32, tag="s2n")
                            nc.scalar.copy(s2n_sb, pst2)
                            nc.sync.dma_start(
                                out=sc2_d[j, :, bass.ts(s, 4)],
                                in_=s2n_sb.rearrange(
                                    "p (t e a) -> p t e a", t=4, e=2
                                ),
                            )

                # ================= out projection =================
                with (
                    tc.tile_pool(name="wot", bufs=1) as wotp,
                    tc.tile_pool(name="outsb", bufs=3) as outp,
                    tc.tile_pool(name="ps_o", bufs=2, space="PSUM") as ps_op,
                ):
                    wot_sb = wotp.tile([128, NOT, DM], C2_DT)
                    nc.sync.dma_start(out=wot_sb, in_=WoT_d[:])
                    bo_sb = wotp.tile([128, NOT], F32)
                    nc.sync.dma_start(out=bo_sb, in_=bo_d[:])
                    for ot in range(NOT):
                        for s in range(NS2):
                            sl = bass.ts(s, 512)
                            pso = ps_op.tile([128, 512], F32)
                            for j in range(NG):
                                nc.tensor.matmul(
                                    pso,
                                    wot_sb[:, j, bass.ts(ot, 128)],
                                    c2T_sb[:, j, sl],
                                    start=(j == 0), stop=(j == NG - 1),
                                )
                            osb = outp.tile([128, 512], F32)
                            nc.scalar.activation(
                                osb, pso, AF.Identity,
                                bias=bo_sb[:, ot : ot + 1], scale=1.0,
                            )
                            nc.sync.dma_start(out=outT_d[ot, :, sl], in_=osb)

    nc.compile()
    return nc


_program_cache = {}


def get_program():
    if "nc" not in _program_cache:
        _program_cache["nc"] = build_program()
    return _program_cache["nc"]


def make_in_maps(q, k, v, Wq, bq, Wk, bk, Wv, bv, Wo, bo, agent_tokens):
    """Host-side sharding/packing. All inputs np.float32 full tensors."""
    u_np = mybir.dt.np(U_DT)
    scale = np.float32(1.0 / np.sqrt(HS))
    q_ag = agent_tokens[0] @ Wq.T + bq            # [NH, A, HS]
    q_eff = (q_ag @ Wk) * scale                   # [NH, A, HS]
    c_q = (q_ag @ bk) * scale                     # [NH, A]

    # block-diagonal q_effT: [g, 0:64, 0:64] = q_eff[2g].T etc.
    qeT = np.zeros((NG, 128, 128), dtype=np.float32)
    for g in range(NG):
        qeT[g, 0:64, 0:64] = q_eff[2 * g].T
        qeT[g, 64:128, 64:128] = q_eff[2 * g + 1].T
    cq_p = np.ascontiguousarray(c_q.reshape(NG, 128).T).astype(np.float32)
    WoT = np.ascontiguousarray(
        Wo.T.reshape(NOT, 128, DM).transpose(1, 0, 2)
    ).astype(mybir.dt.np(C2_DT))
    Wvbd = np.zeros((128, 128), dtype=np.float32)
    Wvbd[0:64, 0:64] = Wv.T
    Wvbd[64:128, 64:128] = Wv.T
    bv_p = np.tile(bv.reshape(1, HS), (2, 1)).reshape(128, 1).astype(np.float32)
    bo_p = np.ascontiguousarray(bo.reshape(NOT, 128).T).astype(np.float32)
    ident = np.eye(128, dtype=np.float32)
    onesb = np.zeros((128, 128), dtype=np.float32)
    onesb[0:64, 0:64] = 1.0
    onesb[64:128, 64:128] = 1.0
    identh = np.eye(128, dtype=u_np)

    in_maps = []
    for c in range(NCORES):
        b = c // 2
        half = c % 2                    # 0: head-pairs 0..3, 1: 4..7
        nlo = half * NLOC
        gsl = slice(half * NGO, half * NGO + NGO)
        hsl = slice(half * 2 * NGO, half * 2 * NGO + 2 * NGO)
        kT_c = np.ascontiguousarray(
            k[b, hsl].transpose(0, 2, 1).reshape(NGO, 128, N)
        ).astype(np.float32)
        # v pairs: [g, p, t, 0:64] = v[2g, t*128+p, :], [64:128] = v[2g+1]
        v_r = v[b, hsl].reshape(2 * NGO, NT1, 128, HS)   # [h, t, p, d]
        v_c = np.empty((NGO, 128, NT1, 128), dtype=u_np)
        v_c[:, :, :, 0:64] = v_r[0::2].transpose(0, 2, 1, 3)
        v_c[:, :, :, 64:128] = v_r[1::2].transpose(0, 2, 1, 3)
        qT_c = np.ascontiguousarray(
            (q[b, :, nlo : nlo + NLOC, :] * scale)
            .transpose(0, 2, 1)
            .reshape(NG, 128, NLOC)
        ).astype(np.float32)
        in_maps.append(
            {
                "kT": kT_c,
                "v": v_c,
                "qT": qT_c,
                "WoT": WoT,
                "qeT": np.ascontiguousarray(qeT[gsl]),
                "cq": np.ascontiguousarray(cq_p[:, gsl]),
                "Wvbd": Wvbd,
                "bv": bv_p,
                "bo": bo_p,
                "ident": ident,
                "onesb": onesb,
                "identh": identh,
            }
        )
    return in_maps


def assemble_outputs(results):
    """results: list of 8 dicts with 'outT' [NOT,128,NLOC] and
    'sc2' [NG,128,NT2,2,A]. Returns (output, scores2)."""
    output = np.empty((B, N, DM), dtype=np.float32)
    scores2 = np.empty((B, NH, N, A), dtype=np.float32)
    for c in range(NCORES):
        b = c // 2
        nlo = (c % 2) * NLOC
        outT = results[c]["outT"]                  # [NOT, 128, NLOC]
        output[b, nlo : nlo + NLOC, :] = outT.reshape(DM, NLOC).T
        sc2 = results[c]["sc2"]                    # [NG, 128, NT2, 2, A]
        scores2[b, :, nlo : nlo + NLOC, :] = (
            sc2.transpose(0, 3, 2, 1, 4).reshape(NH, NLOC, A)
        )
    return output, scores2


def kernel(**inputs):
    inputs = {k_: np.asarray(v_, dtype=np.float32) for k_, v_ in inputs.items()}
    nc = get_program()
    in_maps = make_in_maps(**inputs)
    res = run_bass_kernel_spmd(nc, in_maps, list(range(NCORES)))
    return assemble_outputs(res.results)
